# revision 1
# baseline (speedup 1.0000x reference)
"""Causal single-head attention (B=16, S=2048, D=1024, H=64) on 8 TRN2 cores.

Sharding: data-parallel over batch (2 per core); weights replicated.

Per-core Bass/Tile kernel, for each local batch:
  1. x is cast fp32->bf16 during the SWDGE load DMA, then transposed into
     per-s-tile xT tiles via one DMA-XBAR transpose each (a 3D out AP's
     middle dim extends the partition dim, so one instruction transposes a
     whole [128, D] tile).  The PE never touches the transposition.
  2. Projections on PE with packed weights [Wq/H | Wk] (M=128) and Wv:
     per-s-chunk qT/kT/vT tiles in [H, S-chunk] layout, which is exactly
     what the scores matmul needs (contraction over H on partitions).
  3. scoresT[sk, sq] per (key-block, s-chunk) tile, causal chunks only;
     exp() is applied by ScalarE directly PSUM->SBUF(bf16).  No
     max-subtraction: scores = q.k/H are bounded (|s| < ~1) so exp cannot
     overflow, and softmax is shift-invariant so the result matches the
     reference.  The diagonal block gets a multiplicative triangular mask.
  4. out = attn @ [v | 1]: the ones-column appended to v accumulates the
     softmax denominator for free in PSUM; reciprocal+scale normalizes.

All tiles are sized to the producer/consumer granularity (per s-tile /
s-chunk / attention chunk) so Tile's dependency tracking pipelines the
phases instead of serializing them at phase boundaries.
"""

import sys

import numpy as np

if "/opt/trn_rl_repo" not in sys.path:
    sys.path.insert(0, "/opt/trn_rl_repo")

import concourse.mybir as mybir  # noqa: E402
import concourse.tile as tile  # noqa: E402
from concourse import bacc  # noqa: E402
from concourse.bass_utils import run_bass_kernel_spmd  # noqa: E402
from concourse.masks import make_upper_triangular  # noqa: E402

F32 = mybir.dt.float32
BF16 = mybir.dt.bfloat16
AF = mybir.ActivationFunctionType

B, S, D, H = 16, 2048, 1024, 64
N_CORES = 8
B_PER_CORE = B // N_CORES


def _build_kernel(B_per_core: int, S: int, D: int, H: int):
    assert D % 128 == 0 and S % 512 == 0 and H == 64
    DC = D // 128          # d-chunks of 128
    ST = S // 128          # s-tiles of 128 (== key blocks)
    SC = S // 512          # s-chunks of 512
    KB = ST

    nc = bacc.Bacc("TRN2", target_bir_lowering=False, debug=False,
                   num_devices=N_CORES)
    x_in = nc.dram_tensor("x", [B_per_core, S, D], F32, kind="ExternalInput")
    wq_in = nc.dram_tensor("Wq", [D, H], F32, kind="ExternalInput")
    wk_in = nc.dram_tensor("Wk", [D, H], F32, kind="ExternalInput")
    wv_in = nc.dram_tensor("Wv", [D, H], F32, kind="ExternalInput")
    out_dram = nc.dram_tensor("out", [B_per_core, S, H], F32,
                              kind="ExternalOutput")

    SCC = S // 1024        # wide s-chunks of 1024 (scores/exp granularity)
    n_attn_chunks = sum(SCC - kb // 8 for kb in range(KB))  # 24 at S=2048

    with tile.TileContext(nc) as tc:
        with (
            tc.tile_pool(name="consts", bufs=1) as consts,
            tc.tile_pool(name="xbf", bufs=4) as xbf_pool,
            tc.tile_pool(name="xt", bufs=2 * SC) as xt_pool,
            tc.tile_pool(name="qkvt", bufs=2 * SCC) as qkvt_pool,
            tc.tile_pool(name="vsb", bufs=2) as vsb_pool,
            tc.tile_pool(name="attnt", bufs=n_attn_chunks + 4) as attnt_pool,
            tc.tile_pool(name="outp", bufs=4) as out_pool,
            tc.tile_pool(name="pp", bufs=2, space="PSUM") as proj_psum,
            tc.tile_pool(name="sp", bufs=2, space="PSUM") as scores_psum,
            tc.tile_pool(name="ap", bufs=2, space="PSUM") as av_psum,
        ):
            # wqk: cols 0:64 = Wq * (1/H) (folds the score scale), 64:128 = Wk
            wqk = consts.tile([128, DC, 128], BF16)
            wv = consts.tile([128, DC, H], BF16)
            nc.gpsimd.dma_start(
                out=wqk[:, :, 0:H],
                in_=wq_in.rearrange("(c p) h -> p c h", p=128))
            nc.gpsimd.dma_start(
                out=wqk[:, :, H:128],
                in_=wk_in.rearrange("(c p) h -> p c h", p=128))
            nc.gpsimd.dma_start(
                out=wv[:],
                in_=wv_in.rearrange("(c p) h -> p c h", p=128))
            nc.vector.tensor_scalar_mul(wqk[:, :, 0:H], wqk[:, :, 0:H],
                                        1.0 / H)
            # mask[i, j] = 1.0 where j >= i (valid: sq_local >= sk_local)
            mask = consts.tile([128, 128], BF16)
            make_upper_triangular(nc, mask[:], val=1.0, diag=True)

            for b in range(B_per_core):
                # ---- load + cast + transpose x (per s-chunk) ----
                # one SWDGE cast-load + one 4096-wide XBAR transpose per
                # 512-row s-chunk: the transpose's source column-block e
                # = st*DC + dc maps to out offset st*128 + dc*512, i.e.
                # xt's [dc, st-within-chunk] layout, expressed as a 4D
                # out AP (extra dims extend the partition dim in order).
                xts = []
                for sc in range(SC):
                    xbf = xbf_pool.tile([128, 4, D], BF16, tag="xbf")
                    nc.gpsimd.dma_start(
                        out=xbf[:],
                        in_=x_in[b, sc * 512:(sc + 1) * 512, :].rearrange(
                            "(st p) d -> p st d", p=128))
                    # transpose-natural layout: [128, e, 128] with
                    # e = st*DC + dc (contiguous out, 3D)
                    xt = xt_pool.tile([128, 4 * DC, 128], BF16, tag="xt")
                    nc.sync.dma_start(out=xt[:], in_=xbf[:], transpose=True)
                    xts.append(
                        xt[:].rearrange("p (st dc) s -> p dc st s", dc=DC))

                # ---- projections (per s-chunk of 512) ----
                qTs, kTs, vTs = [], [], []
                v_sb = vsb_pool.tile([128, KB, 80], BF16)
                # fill with 1.0; v transposes overwrite cols 0:H, leaving
                # col H == 1.0 (the softmax-denominator column)
                nc.vector.memset(v_sb[:], 1.0)
                for sc in range(SC):
                    if sc % 2 == 0:
                        qT = qkvt_pool.tile([64, 1024], BF16, tag="qT")
                        kT = qkvt_pool.tile([64, 1024], BF16, tag="kT")
                        vT = qkvt_pool.tile([64, 1024], BF16, tag="vT")
                        qTs.append(qT)
                        kTs.append(kT)
                        vTs.append(vT)
                    hs = slice((sc % 2) * 512, (sc % 2) * 512 + 512)
                    ps = proj_psum.tile([128, 512], F32, tag="proj")
                    for dc in range(DC):
                        nc.tensor.matmul(
                            ps[:], lhsT=wqk[:, dc, :],
                            rhs=xts[sc][:, dc, :, :],
                            start=(dc == 0), stop=(dc == DC - 1))
                    nc.vector.tensor_copy(qT[:, hs], ps[0:64, :])
                    nc.vector.tensor_copy(kT[:, hs], ps[64:128, :])
                    ps2 = proj_psum.tile([64, 512], F32, tag="proj")
                    for dc in range(DC):
                        nc.tensor.matmul(
                            ps2[:], lhsT=wv[:, dc, :],
                            rhs=xts[sc][:, dc, :, :],
                            start=(dc == 0), stop=(dc == DC - 1))
                    nc.vector.tensor_copy(vT[:, hs], ps2[:])
                    nc.sync.dma_start(
                        out=v_sb[:, sc * 4:(sc + 1) * 4, 0:H],
                        in_=vT[:, hs], transpose=True)

                # ---- attention phase 1: attnT chunks = exp(scoresT) ----
                # chunk (kb, scc): sk-block kb x sq [scc*1024, (scc+1)*1024)
                # scores psum is bf16 (one bank holds 1024 bf16), N=1024
                attn = {}
                for scc in range(SCC):
                    for kb in range((scc + 1) * 8):
                        k0 = kb * 128
                        kt_sc, kt_off = kTs[kb // 8], k0 - (kb // 8) * 1024
                        ps = scores_psum.tile([128, 1024], F32,
                                              tag="scores")
                        for h in range(2):
                            nc.tensor.matmul(
                                ps[:, h * 512:(h + 1) * 512],
                                lhsT=kt_sc[:, kt_off:kt_off + 128],
                                rhs=qTs[scc][:, h * 512:(h + 1) * 512],
                                start=True, stop=True)
                        at = attnt_pool.tile([128, 1024], BF16, tag="at")
                        nc.scalar.activation(out=at[:], in_=ps[:],
                                             func=AF.Exp)
                        if kb // 8 == scc:
                            # diagonal block: zero sq_local < sk_local
                            d0 = k0 - scc * 1024
                            nc.vector.tensor_mul(
                                at[:, d0:d0 + 128], at[:, d0:d0 + 128],
                                mask[:])
                        attn[(kb, scc)] = at

                # ---- attention phase 2: out = (attn @ [v|1]) normalized --
                for qb in range(ST):
                    po = av_psum.tile([128, H + 1], F32, tag="av")
                    q0, scc = qb * 128, qb // 8
                    qoff = q0 - scc * 1024
                    for kb in range(qb + 1):
                        nc.tensor.matmul(
                            po[:],
                            lhsT=attn[(kb, scc)][:, qoff:qoff + 128],
                            rhs=v_sb[:, kb, 0:H + 1],
                            start=(kb == 0), stop=(kb == qb))
                    recip = out_pool.tile([128, 1], F32, tag="recip")
                    nc.vector.reciprocal(recip[:], po[:, H:H + 1])
                    out_t = out_pool.tile([128, H], F32, tag="out")
                    nc.vector.tensor_scalar_mul(out_t[:], po[:, 0:H],
                                                recip[:])
                    nc.sync.dma_start(
                        out=out_dram[b, q0:q0 + 128, :], in_=out_t[:])

    nc.compile()
    return nc


_NC_CACHE = {}


def _get_nc():
    key = (B_PER_CORE, S, D, H)
    if key not in _NC_CACHE:
        _NC_CACHE[key] = _build_kernel(*key)
    return _NC_CACHE[key]


def kernel(x: np.ndarray, Wq: np.ndarray, Wk: np.ndarray, Wv: np.ndarray):
    """Full-input entry point: shards over batch, runs 8 cores, gathers."""
    assert x.shape == (B, S, D)
    nc = _get_nc()
    core_ids = list(range(N_CORES))
    x = np.ascontiguousarray(np.asarray(x, dtype=np.float32))
    Wq = np.ascontiguousarray(np.asarray(Wq, dtype=np.float32))
    Wk = np.ascontiguousarray(np.asarray(Wk, dtype=np.float32))
    Wv = np.ascontiguousarray(np.asarray(Wv, dtype=np.float32))
    in_maps = [
        {"x": x[c * B_PER_CORE:(c + 1) * B_PER_CORE], "Wq": Wq, "Wk": Wk,
         "Wv": Wv}
        for c in core_ids
    ]
    res = run_bass_kernel_spmd(nc, in_maps, core_ids)
    return np.concatenate([res.results[c]["out"] for c in core_ids], axis=0)



# revision 6
# speedup vs baseline: 5.4402x; 5.4402x over previous
"""Causal single-head attention (B=16, S=2048, D=1024, H=64) on 8 TRN2 cores.

Sharding: data-parallel over batch (2 per core); weights replicated.

Per-core Bass/Tile kernel v4 — software-pipelined at 512-row granularity;
XBAR-transpose count minimized to 4 (each one serializes the DMA stream):

  1. x is cast fp32->bf16 during the SWDGE load DMA (both batches into one
     staging tile), then ONE DMA-XBAR transpose per 512-row chunk yields
     xT tiles for both batches.
  2. Projections on PE: q|k with packed weights [Wq | Wk] (M=128) into
     qT/kT tiles holding BOTH batches (rows 0:64 = b0, 64:128 = b1); the
     1/H score scale is folded into the exp activation's scale.  v is
     computed DIRECTLY in [sk, h] layout (lhsT = xT block, rhs = Wv) so
     no v transpose is needed; a DVE copy drops it into v_sb whose
     columns 64:128 are pre-set to 1.0 (denominator ones-block).
  3. Right after s-chunk q2's projections, all scores chunks (kb, q2) are
     emitted: row-tiled matmuls compute both batches CONCURRENTLY on the
     two K=64 halves of the PE array into one [128, 1024] PSUM tile; ONE
     3-D-AP exp covers both batches; only columns sq >= kb*128 are
     computed (causal trim); the diagonal 128-block gets a multiplicative
     triangular mask (doubled mask, both batches in one DVE op).
  4. AV transposed: out^T[0:128, sq-512] = sum_kb [v|1s]_kb^T @ attnT.
     PSUM rows 64:128 hold the softmax denominator REPLICATED across 64
     partitions (free broadcast); reciprocal_approx_fast + tensor_mul
     normalize.  out^T is written to DRAM as-is (bf16); the host
     transposes/upcasts — values identical to a device-side pass.
"""

import sys

import numpy as np

if "/opt/trn_rl_repo" not in sys.path:
    sys.path.insert(0, "/opt/trn_rl_repo")

import concourse.mybir as mybir  # noqa: E402
import concourse.tile as tile  # noqa: E402
from concourse import bacc  # noqa: E402
from concourse.bass_utils import run_bass_kernel_spmd  # noqa: E402
from concourse.masks import make_upper_triangular  # noqa: E402

F32 = mybir.dt.float32
BF16 = mybir.dt.bfloat16
AF = mybir.ActivationFunctionType

B, S, D, H = 16, 2048, 1024, 64
N_CORES = 8
B_PER_CORE = B // N_CORES


def _build_kernel(B_per_core: int, S: int, D: int, H: int):
    assert D % 128 == 0 and S % 1024 == 0 and H == 64 and B_per_core == 2
    DC = D // 128          # d-chunks of 128
    SC = S // 512          # s-chunks of 512 (pipeline granularity)
    SCC = S // 1024        # output sq-chunks of 1024
    KB = S // 128          # 128-row key blocks

    nc = bacc.Bacc("TRN2", target_bir_lowering=False, debug=False,
                   num_devices=N_CORES)
    x_in = nc.dram_tensor("x", [B_per_core, S, D], F32, kind="ExternalInput")
    wq_in = nc.dram_tensor("Wq", [D, H], F32, kind="ExternalInput")
    wk_in = nc.dram_tensor("Wk", [D, H], F32, kind="ExternalInput")
    wv_in = nc.dram_tensor("Wv", [D, H], F32, kind="ExternalInput")
    # out[b, h, s]: TRANSPOSED unnormalized-layout output (host transposes)
    out_dram = nc.dram_tensor("out", [B_per_core, H, S], BF16,
                              kind="ExternalOutput")

    with tile.TileContext(nc) as tc:
        with (
            tc.tile_pool(name="consts", bufs=1) as consts,
            tc.tile_pool(name="xbf", bufs=3) as xbf_pool,
            tc.tile_pool(name="xt", bufs=3) as xt_pool,
            tc.tile_pool(name="attnt", bufs=20) as attnt_pool,
            tc.tile_pool(name="fin", bufs=2) as fin_pool,
            tc.tile_pool(name="mm", bufs=4, space="PSUM") as mm_psum,
            tc.tile_pool(name="sc", bufs=2, space="PSUM") as sc_psum,
        ):
            # ---- constants ----
            wqk = consts.tile([128, DC, 128], BF16)
            wv = consts.tile([128, DC, H], BF16)
            nc.gpsimd.dma_start(
                out=wqk[:, :, 0:H],
                in_=wq_in.rearrange("(c p) h -> p c h", p=128))
            nc.gpsimd.dma_start(
                out=wqk[:, :, H:128],
                in_=wk_in.rearrange("(c p) h -> p c h", p=128))
            nc.gpsimd.dma_start(
                out=wv[:],
                in_=wv_in.rearrange("(c p) h -> p c h", p=128))
            # mask2[i, *, j] = 1.0 where j >= i, doubled for the two batches
            mask2 = consts.tile([128, 2, 128], BF16)
            make_upper_triangular(nc, mask2[:, 0, :], val=1.0, diag=True)
            make_upper_triangular(nc, mask2[:, 1, :], val=1.0, diag=True)

            # both batches packed: rows 0:64 = b0, 64:128 = b1
            qT01 = consts.tile([128, S], BF16)
            kT01 = consts.tile([128, S], BF16)
            # v_sb[b]: [sk_local, kb, 0:64]=v, [.., 64:128]=1.0 (denominator
            # broadcast columns)
            v_sb = [consts.tile([128, KB, 128], BF16, name=f"vsb{b}")
                    for b in range(B_per_core)]
            for b in range(B_per_core):
                nc.vector.memset(v_sb[b][:, :, H:128], 1.0)
            # onrm[b]: normalized out^T halves awaiting the DRAM write
            onrm = [consts.tile([64, 1024], BF16, name=f"onrm{b}")
                    for b in range(B_per_core)]

            for q2 in range(SC):
                hs = slice(q2 * 512, q2 * 512 + 512)
                # ---- load (both batches) + ONE transpose for s-chunk q2 --
                xbf = xbf_pool.tile([128, 2, 4, D], BF16, tag="xbf")
                for b in range(B_per_core):
                    nc.gpsimd.dma_start(
                        out=xbf[:, b],
                        in_=x_in[b, hs, :].rearrange(
                            "(st p) d -> p st d", p=128))
                xt = xt_pool.tile([128, 2 * 4 * DC, 128], BF16, tag="xt")
                nc.sync.dma_start(out=xt[:], in_=xbf[:], transpose=True)
                # e = (b*4+st)*DC + dc  ->  [p, b, st, dc, s]
                xtv = xt[:].rearrange("p (b st dc) s -> p b st dc s",
                                      b=2, dc=DC)

                # ---- projections ----
                for b in range(B_per_core):
                    ps = mm_psum.tile([128, 512], F32, tag="mm")
                    for dc in range(DC):
                        nc.tensor.matmul(
                            ps[:], lhsT=wqk[:, dc, :],
                            rhs=xtv[:, b, :, dc, :],
                            start=(dc == 0), stop=(dc == DC - 1))
                    rows = slice(64 * b, 64 * b + 64)
                    nc.vector.tensor_copy(qT01[rows, hs], ps[0:64, :])
                    nc.vector.tensor_copy(kT01[rows, hs], ps[64:128, :])

                    # v directly in [sk, h] layout: lhsT = xT block
                    psv = mm_psum.tile([128, 4, H], F32, tag="mm")
                    for j in range(4):
                        for dc in range(DC):
                            nc.tensor.matmul(
                                psv[:, j, :], lhsT=xtv[:, b, j, dc, :],
                                rhs=wv[:, dc, :],
                                start=(dc == 0), stop=(dc == DC - 1))
                    nc.vector.tensor_copy(
                        v_sb[b][:, q2 * 4:q2 * 4 + 4, 0:H], psv[:])

                # ---- scores for sq-chunk q2, both batches packed ----
                attn = {}
                for kb in range(4 * q2 + 4):
                    diag = (kb // 4 == q2)
                    d0 = (kb - 4 * q2) * 128 if diag else 0
                    kcols = slice(kb * 128, kb * 128 + 128)
                    psAB = sc_psum.tile([128, 1024], F32, tag="sc")
                    for b in range(B_per_core):
                        rows = slice(64 * b, 64 * b + 64)
                        nc.tensor.matmul(
                            psAB[:, b * 512 + d0:b * 512 + 512],
                            lhsT=kT01[rows, kcols],
                            rhs=qT01[rows, q2 * 512 + d0:q2 * 512 + 512],
                            start=True, stop=True)
                    at = attnt_pool.tile([128, 1024], BF16, tag="at")
                    atv = at[:].rearrange("p (b c) -> p b c", b=2)
                    psv2 = psAB[:].rearrange("p (b c) -> p b c", b=2)
                    nc.scalar.activation(out=atv[:, :, d0:512],
                                         in_=psv2[:, :, d0:512],
                                         func=AF.Exp, scale=1.0 / H)
                    if diag:
                        nc.vector.tensor_mul(
                            atv[:, :, d0:d0 + 128], atv[:, :, d0:d0 + 128],
                            mask2[:])
                    attn[kb] = at

                # ---- AV + finish for sq-chunk q2 ----
                for b in range(B_per_core):
                    pav = mm_psum.tile([128, 512], F32, tag="mm")
                    for kb in range(4 * q2 + 4):
                        diag = (kb // 4 == q2)
                        d0 = (kb - 4 * q2) * 128 if diag else 0
                        nc.tensor.matmul(
                            pav[:, d0:512],
                            lhsT=v_sb[b][:, kb, :],
                            rhs=attn[kb][:, b * 512 + d0:b * 512 + 512],
                            start=(kb == 0), stop=(kb == 4 * q2 + 3))
                    rc = fin_pool.tile([64, 512], F32, tag="rc")
                    nc.vector.reciprocal(rc[:], pav[64:128, :])
                    half = slice((q2 % 2) * 512, (q2 % 2) * 512 + 512)
                    nc.vector.tensor_mul(onrm[b][:, half], pav[0:64, :],
                                         rc[:])
                    if q2 % 2 == 1:
                        scc = q2 // 2
                        nc.sync.dma_start(
                            out=out_dram[b, :, scc * 1024:scc * 1024 + 1024],
                            in_=onrm[b][:])

    nc.compile()
    return nc


_NC_CACHE = {}


def _get_nc():
    key = (B_PER_CORE, S, D, H)
    if key not in _NC_CACHE:
        _NC_CACHE[key] = _build_kernel(*key)
    return _NC_CACHE[key]


def kernel(x: np.ndarray, Wq: np.ndarray, Wk: np.ndarray, Wv: np.ndarray):
    """Full-input entry point: shards over batch, runs 8 cores, gathers."""
    assert x.shape == (B, S, D)
    nc = _get_nc()
    core_ids = list(range(N_CORES))
    x = np.ascontiguousarray(np.asarray(x, dtype=np.float32))
    Wq = np.ascontiguousarray(np.asarray(Wq, dtype=np.float32))
    Wk = np.ascontiguousarray(np.asarray(Wk, dtype=np.float32))
    Wv = np.ascontiguousarray(np.asarray(Wv, dtype=np.float32))
    in_maps = [
        {"x": x[c * B_PER_CORE:(c + 1) * B_PER_CORE], "Wq": Wq, "Wk": Wk,
         "Wv": Wv}
        for c in core_ids
    ]
    res = run_bass_kernel_spmd(nc, in_maps, core_ids)
    outs = []
    for c in core_ids:
        o = np.asarray(res.results[c]["out"])  # [2, H, S] bf16, transposed
        outs.append(o.astype(np.float32).transpose(0, 2, 1))
    return np.concatenate(outs, axis=0)


# revision 10
# speedup vs baseline: 47.6963x; 8.7674x over previous
"""Causal single-head attention (B=16, S=2048, D=1024, H=64) on 8 TRN2 cores.

Sharding: data-parallel over batch (2 per core); weights replicated.

Per-core Bass/Tile kernel v4 — software-pipelined at 512-row granularity;
XBAR-transpose count minimized to 4 (each one serializes the DMA stream):

  1. x is cast fp32->bf16 during the SWDGE load DMA (both batches into one
     staging tile), then ONE DMA-XBAR transpose per 512-row chunk yields
     xT tiles for both batches.
  2. Projections on PE: q|k with packed weights [Wq | Wk] (M=128) into
     qT/kT tiles holding BOTH batches (rows 0:64 = b0, 64:128 = b1); the
     1/H score scale is folded into the exp activation's scale.  v is
     computed DIRECTLY in [sk, h] layout (lhsT = xT block, rhs = Wv) so
     no v transpose is needed; a DVE copy drops it into v_sb whose
     columns 64:128 are pre-set to 1.0 (denominator ones-block).
  3. Right after s-chunk q2's projections, all scores chunks (kb, q2) are
     emitted: row-tiled matmuls compute both batches CONCURRENTLY on the
     two K=64 halves of the PE array into one [128, 1024] PSUM tile; ONE
     3-D-AP exp covers both batches; only columns sq >= kb*128 are
     computed (causal trim); the diagonal 128-block gets a multiplicative
     triangular mask (doubled mask, both batches in one DVE op).
  4. AV transposed: out^T[0:128, sq-512] = sum_kb [v|1s]_kb^T @ attnT.
     PSUM rows 64:128 hold the softmax denominator REPLICATED across 64
     partitions (free broadcast); reciprocal_approx_fast + tensor_mul
     normalize.  out^T is written to DRAM as-is (bf16); the host
     transposes/upcasts — values identical to a device-side pass.
"""

import sys

import numpy as np

if "/opt/trn_rl_repo" not in sys.path:
    sys.path.insert(0, "/opt/trn_rl_repo")

import concourse.mybir as mybir  # noqa: E402
import concourse.tile as tile  # noqa: E402
from concourse import bacc  # noqa: E402
from concourse.bass_utils import run_bass_kernel_spmd  # noqa: E402
from concourse.masks import make_upper_triangular  # noqa: E402

F32 = mybir.dt.float32
BF16 = mybir.dt.bfloat16
AF = mybir.ActivationFunctionType

B, S, D, H = 16, 2048, 1024, 64
N_CORES = 8
B_PER_CORE = B // N_CORES


def _build_kernel(B_per_core: int, S: int, D: int, H: int, repeat: int = 1):
    assert D % 128 == 0 and S % 1024 == 0 and H == 64 and B_per_core == 2
    DC = D // 128          # d-chunks of 128
    SC = S // 512          # s-chunks of 512 (pipeline granularity)
    SCC = S // 1024        # output sq-chunks of 1024
    KB = S // 128          # 128-row key blocks

    nc = bacc.Bacc("TRN2", target_bir_lowering=False, debug=False,
                   num_devices=N_CORES)
    x_in = nc.dram_tensor("x", [B_per_core, S, D], F32, kind="ExternalInput")
    wq_in = nc.dram_tensor("Wq", [D, H], F32, kind="ExternalInput")
    wk_in = nc.dram_tensor("Wk", [D, H], F32, kind="ExternalInput")
    wv_in = nc.dram_tensor("Wv", [D, H], F32, kind="ExternalInput")
    # out[b, h, s]: TRANSPOSED layout output (host transposes);
    # leading repeat dim only used by the dev timing proxy
    out_dram_r = nc.dram_tensor("out", [repeat, B_per_core, H, S], BF16,
                                kind="ExternalOutput")

    with tile.TileContext(nc) as tc:
        with (
            tc.tile_pool(name="consts", bufs=1) as consts,
            tc.tile_pool(name="xbf", bufs=4) as xbf_pool,
            tc.tile_pool(name="xt", bufs=3) as xt_pool,
            tc.tile_pool(name="attnt", bufs=22) as attnt_pool,
            tc.tile_pool(name="fin", bufs=2) as fin_pool,
            tc.tile_pool(name="mm", bufs=4, space="PSUM") as mm_psum,
            tc.tile_pool(name="sc", bufs=2, space="PSUM") as sc_psum,
        ):
            # ---- constants ----
            wqk = consts.tile([128, DC, 128], BF16)
            wv = consts.tile([128, DC, H], BF16)
            nc.gpsimd.dma_start(
                out=wqk[:, :, 0:H],
                in_=wq_in.rearrange("(c p) h -> p c h", p=128))
            nc.gpsimd.dma_start(
                out=wqk[:, :, H:128],
                in_=wk_in.rearrange("(c p) h -> p c h", p=128))
            nc.gpsimd.dma_start(
                out=wv[:],
                in_=wv_in.rearrange("(c p) h -> p c h", p=128))
            # mask2[i, *, j] = 1.0 where j >= i, doubled for the two batches
            mask2 = consts.tile([128, 2, 128], BF16)
            make_upper_triangular(nc, mask2[:, 0, :], val=1.0, diag=True)
            make_upper_triangular(nc, mask2[:, 1, :], val=1.0, diag=True)

            # both batches packed: rows 0:64 = b0, 64:128 = b1
            qT01 = consts.tile([128, S], BF16)
            kT01 = consts.tile([128, S], BF16)
            # v_sb[b]: [sk_local, kb, 0:64]=v, [.., 64:128]=1.0 (denominator
            # broadcast columns)
            v_sb = [consts.tile([128, KB, 128], BF16, name=f"vsb{b}")
                    for b in range(B_per_core)]
            for b in range(B_per_core):
                nc.vector.memset(v_sb[b][:, :, H:128], 1.0)
            # onrm[b]: normalized out^T halves awaiting the DRAM write
            onrm = [consts.tile([64, 1024], BF16, name=f"onrm{b}")
                    for b in range(B_per_core)]

            for rep in range(repeat):
              out_dram = out_dram_r[rep]
              for q2 in range(SC):
                hs = slice(q2 * 512, q2 * 512 + 512)
                # ---- load (both batches) + ONE transpose for s-chunk q2 --
                xbf = xbf_pool.tile([128, 2, 4, D], BF16, tag="xbf")
                for b in range(B_per_core):
                    nc.gpsimd.dma_start(
                        out=xbf[:, b],
                        in_=x_in[b, hs, :].rearrange(
                            "(st p) d -> p st d", p=128))
                xt = xt_pool.tile([128, 2 * 4 * DC, 128], BF16, tag="xt")
                nc.sync.dma_start(out=xt[:], in_=xbf[:], transpose=True)
                # e = (b*4+st)*DC + dc  ->  [p, b, st, dc, s]
                xtv = xt[:].rearrange("p (b st dc) s -> p b st dc s",
                                      b=2, dc=DC)

                # ---- projections: q|k for both batches FIRST (scores
                # depend on them), v after (only AV needs it) ----
                for b in range(B_per_core):
                    ps = mm_psum.tile([128, 512], F32, tag="mm")
                    for dc in range(DC):
                        nc.tensor.matmul(
                            ps[:], lhsT=wqk[:, dc, :],
                            rhs=xtv[:, b, :, dc, :],
                            start=(dc == 0), stop=(dc == DC - 1))
                    rows = slice(64 * b, 64 * b + 64)
                    nc.vector.tensor_copy(qT01[rows, hs], ps[0:64, :])
                    nc.vector.tensor_copy(kT01[rows, hs], ps[64:128, :])
                for b in range(B_per_core):
                    # v directly in [sk, h] layout: lhsT = xT block
                    psv = mm_psum.tile([128, 4, H], F32, tag="mm")
                    for j in range(4):
                        for dc in range(DC):
                            nc.tensor.matmul(
                                psv[:, j, :], lhsT=xtv[:, b, j, dc, :],
                                rhs=wv[:, dc, :],
                                start=(dc == 0), stop=(dc == DC - 1))
                    nc.vector.tensor_copy(
                        v_sb[b][:, q2 * 4:q2 * 4 + 4, 0:H], psv[:])

                # ---- scores + AV for sq-chunk q2, AV lagging 2 chunks so
                # PE never waits on the exp of the chunk it consumes ----
                nkb = 4 * q2 + 4
                attn = {}
                pav = [mm_psum.tile([128, 512], F32, tag="mm",
                                    name=f"pav{b}")
                       for b in range(B_per_core)]
                for step in range(nkb + 2):
                    if step < nkb:
                        kb = step
                        diag = (kb // 4 == q2)
                        d0 = (kb - 4 * q2) * 128 if diag else 0
                        kcols = slice(kb * 128, kb * 128 + 128)
                        psAB = sc_psum.tile([128, 1024], F32, tag="sc")
                        for b in range(B_per_core):
                            rows = slice(64 * b, 64 * b + 64)
                            nc.tensor.matmul(
                                psAB[:, b * 512 + d0:b * 512 + 512],
                                lhsT=kT01[rows, kcols],
                                rhs=qT01[rows,
                                         q2 * 512 + d0:q2 * 512 + 512],
                                start=True, stop=True)
                        at = attnt_pool.tile([128, 1024], BF16, tag="at")
                        atv = at[:].rearrange("p (b c) -> p b c", b=2)
                        psv2 = psAB[:].rearrange("p (b c) -> p b c", b=2)
                        nc.scalar.activation(out=atv[:, :, d0:512],
                                             in_=psv2[:, :, d0:512],
                                             func=AF.Exp, scale=1.0 / H)
                        if diag:
                            nc.vector.tensor_mul(
                                atv[:, :, d0:d0 + 128],
                                atv[:, :, d0:d0 + 128], mask2[:])
                        attn[kb] = at
                    if step >= 2:
                        kb = step - 2
                        diag = (kb // 4 == q2)
                        d0 = (kb - 4 * q2) * 128 if diag else 0
                        for b in range(B_per_core):
                            nc.tensor.matmul(
                                pav[b][:, d0:512],
                                lhsT=v_sb[b][:, kb, :],
                                rhs=attn[kb][:, b * 512 + d0:b * 512 + 512],
                                start=(kb == 0), stop=(kb == nkb - 1))

                # ---- finish ----
                for b in range(B_per_core):
                    rc = fin_pool.tile([64, 512], F32, tag="rc")
                    nc.vector.reciprocal(rc[:], pav[b][64:128, :])
                    half = slice((q2 % 2) * 512, (q2 % 2) * 512 + 512)
                    nc.vector.tensor_mul(onrm[b][:, half], pav[b][0:64, :],
                                         rc[:])
                    if q2 % 2 == 1:
                        scc = q2 // 2
                        nc.sync.dma_start(
                            out=out_dram[b, :, scc * 1024:scc * 1024 + 1024],
                            in_=onrm[b][:])

    nc.compile()
    return nc


_NC_CACHE = {}


def _get_nc():
    key = (B_PER_CORE, S, D, H)
    if key not in _NC_CACHE:
        _NC_CACHE[key] = _build_kernel(*key)
    return _NC_CACHE[key]


def kernel(x: np.ndarray, Wq: np.ndarray, Wk: np.ndarray, Wv: np.ndarray):
    """Full-input entry point: shards over batch, runs 8 cores, gathers."""
    assert x.shape == (B, S, D)
    nc = _get_nc()
    core_ids = list(range(N_CORES))
    x = np.ascontiguousarray(np.asarray(x, dtype=np.float32))
    Wq = np.ascontiguousarray(np.asarray(Wq, dtype=np.float32))
    Wk = np.ascontiguousarray(np.asarray(Wk, dtype=np.float32))
    Wv = np.ascontiguousarray(np.asarray(Wv, dtype=np.float32))
    in_maps = [
        {"x": x[c * B_PER_CORE:(c + 1) * B_PER_CORE], "Wq": Wq, "Wk": Wk,
         "Wv": Wv}
        for c in core_ids
    ]
    res = run_bass_kernel_spmd(nc, in_maps, core_ids)
    outs = []
    for c in core_ids:
        o = np.asarray(res.results[c]["out"])[0]  # [2, H, S] bf16
        outs.append(o.astype(np.float32).transpose(0, 2, 1))
    return np.concatenate(outs, axis=0)
